# revision 26
# baseline (speedup 1.0000x reference)
"""Self-contained Trainium2 kernel for nn_GATNetSelectiveResidualsUpdated.

GATv2 layer + MLP head + pairwise-distance output, distributed over 8
NeuronCores: dst-nodes (and the cdist row block) are sharded per core,
edges grouped by dst block, xl table built redundantly per core, y
all-gathered on-device for the cdist columns.

Optimized edge phase: attention logits are computed with accumulate-
reductions instead of PE transposes.  Host folds |att| into Wl/Wr
column scales (with a sign-segregated column permutation), so
e_h = sum_pos lrelu(p) - sum_neg lrelu(p) where p is the scaled
pre-activation; pos groups reduce on DVE (scalar_tensor_tensor accum),
the merged neg region uses one scalar Prelu(scale=-0.2, alpha=5) and
DVE sums; per-head parts combine through exp(a+b)=exp(a)exp(b).
The |att| scale is undone by folding 1/|att| into the first MLP layer
weights (legal because the scales are positive and relu commutes).

cdist uses fp16 hi/lo split aug vectors (K=11) instead of f32 matmuls,
sq_i enters through the Sqrt activation bias (plus eps, replacing the
max(0,.) clamp), and the output is written fp16.

kernel(**inputs) takes the FULL inputs (as produced by setup_inputs())
and returns the FULL [12288, 12288] float32 output.
"""
import numpy as np
import ml_dtypes
from contextlib import ExitStack

import concourse.bass as bass
import concourse.bacc as bacc
import concourse.mybir as mybir
from concourse import tile
from concourse.masks import make_identity

dt = mybir.dt
AF = mybir.ActivationFunctionType
ALU = mybir.AluOpType

N = 12288
F = 512          # H*C
C = 256
NC = 8
ND = N // NC     # 1536 nodes per core
NB = ND // 128   # 12 dst blocks per core
TB = 27          # edge tiles per block (padded)
NM = N // 128    # 96 m-tiles for full table
EPS = 1e-5
EPS2 = 1e-5      # cdist sqrt guard

bf = dt.float16
f32 = dt.float32

_cache = {}


def build_program(phases="full", repeats=1):
    nc = bacc.Bacc(None, target_bir_lowering=False)
    xT = nc.dram_tensor("xT", [F, N], bf, kind="ExternalInput")
    xTl = nc.dram_tensor("xTl", [F, ND], bf, kind="ExternalInput")
    wl = nc.dram_tensor("wl", [F, F], bf, kind="ExternalInput")
    blr = nc.dram_tensor("blr", [1, F], bf, kind="ExternalInput")
    wr = nc.dram_tensor("wr", [F, F], bf, kind="ExternalInput")
    brr = nc.dram_tensor("brr", [1, F], bf, kind="ExternalInput")
    biasc = nc.dram_tensor("biasc", [1, F], bf, kind="ExternalInput")
    sidx = nc.dram_tensor("sidx", [NB, 128, TB], dt.int32, kind="ExternalInput")
    edl_in = nc.dram_tensor("edl_in", [NB, 128, TB], f32, kind="ExternalInput")
    m01_in = nc.dram_tensor("m01_in", [NB, 128, TB, 128], bf, kind="ExternalInput")
    mt_in = nc.dram_tensor("mt_in", [NB, 128, TB, 128], bf, kind="ExternalInput")
    sgn = nc.dram_tensor("sgn", [1, F], bf, kind="ExternalInput")
    # MLP weights
    wda = nc.dram_tensor("wda", [F, 256], bf, kind="ExternalInput")
    bda = nc.dram_tensor("bda", [1, 256], bf, kind="ExternalInput")
    wala = nc.dram_tensor("wala", [F, 256], bf, kind="ExternalInput")
    bala = nc.dram_tensor("bala", [1, 256], bf, kind="ExternalInput")
    wd1 = nc.dram_tensor("wd1", [256, 128], bf, kind="ExternalInput")
    bd1 = nc.dram_tensor("bd1", [1, 128], bf, kind="ExternalInput")
    wal1 = nc.dram_tensor("wal1", [256, 128], bf, kind="ExternalInput")
    bal1 = nc.dram_tensor("bal1", [1, 128], bf, kind="ExternalInput")
    wd2 = nc.dram_tensor("wd2", [128, 64], bf, kind="ExternalInput")
    bd2 = nc.dram_tensor("bd2", [1, 64], bf, kind="ExternalInput")
    wd3 = nc.dram_tensor("wd3", [64, 3], bf, kind="ExternalInput")
    bd3 = nc.dram_tensor("bd3", [1, 3], bf, kind="ExternalInput")
    # LN affine params (f32)
    lnga = nc.dram_tensor("lnga", [1, 256], f32, kind="ExternalInput")
    lnba = nc.dram_tensor("lnba", [1, 256], f32, kind="ExternalInput")
    lng1 = nc.dram_tensor("lng1", [1, 128], f32, kind="ExternalInput")
    lnb1 = nc.dram_tensor("lnb1", [1, 128], f32, kind="ExternalInput")
    lng2 = nc.dram_tensor("lng2", [1, 64], f32, kind="ExternalInput")
    lnb2 = nc.dram_tensor("lnb2", [1, 64], f32, kind="ExternalInput")

    table = nc.dram_tensor("table", [N, 514], bf, kind="Internal")
    AGW = 11  # aug width for cdist
    cc_in = nc.dram_tensor("cc_in", [AGW, ND], bf, kind="Internal")
    cc_out = nc.dram_tensor("cc_out", [NC * AGW, ND], bf, kind="Internal",
                            addr_space="Shared")
    out = nc.dram_tensor("out", [ND, N], bf, kind="ExternalOutput")
    y_out = nc.dram_tensor("y_out", [ND, 3], f32, kind="ExternalOutput")
    tok = nc.dram_tensor("tok", [1, 8], f32, kind="ExternalInput")
    tok_out = nc.dram_tensor("tok_out", [1, 8], f32, kind="ExternalOutput")

    with tile.TileContext(nc) as tc:
      for _rep in range(repeats):
        with ExitStack() as ctx:
            cpool = ctx.enter_context(tc.tile_pool(name="const", bufs=1))
            tok_sb = cpool.tile([1, 8], f32)
            nc.sync.dma_start(out=tok_sb[:], in_=tok[:])
            nc.sync.dma_start(out=tok_out[:], in_=tok_sb[:])
            ident = cpool.tile([128, 128], bf)
            make_identity(nc, ident[:])
            ones1 = cpool.tile([1, 128], bf)
            nc.vector.memset(ones1[:], 1.0)
            eps_sb = cpool.tile([128, 1], f32)
            nc.vector.memset(eps_sb[:], EPS)
            iota_sb = cpool.tile([128, 128], bf)
            nc.gpsimd.iota(iota_sb[:], [[1, 128]], channel_multiplier=0,
                           allow_small_or_imprecise_dtypes=True)
            biasc_sb = cpool.tile([1, F], bf)
            nc.sync.dma_start(out=biasc_sb[:], in_=biasc[:])

            # broadcast constants to all partitions via rank-1 matmuls
            def bcast_f32(row_dram, width, name):
                row_sb = cpool.tile([1, width], f32, name=f"{name}_row")
                nc.sync.dma_start(out=row_sb[:], in_=row_dram[:])
                row_bf = cpool.tile([1, width], bf, name=f"{name}_bf")
                nc.vector.tensor_copy(out=row_bf[:], in_=row_sb[:])
                ps = initps.tile([128, width], f32, tag="initps")
                nc.tensor.matmul(out=ps[:], lhsT=ones1[:], rhs=row_bf[:], start=True, stop=True)
                bcast = cpool.tile([128, width], f32, name=f"{name}_bc")
                nc.vector.tensor_copy(out=bcast[:], in_=ps[:])
                return bcast, row_sb

            sgn_sb = cpool.tile([1, F], bf)
            nc.sync.dma_start(out=sgn_sb[:], in_=sgn[:])
            with tc.tile_pool(name="initps", bufs=2, space="PSUM") as initps:
                ps = initps.tile([128, F], f32, tag="initps")
                nc.tensor.matmul(out=ps[:], lhsT=ones1[:], rhs=biasc_sb[:], start=True, stop=True)
                biasc_bc = cpool.tile([128, F], f32)
                nc.vector.tensor_copy(out=biasc_bc[:], in_=ps[:])
                ps2 = initps.tile([128, F], f32, tag="initps")
                nc.tensor.matmul(out=ps2[:], lhsT=ones1[:], rhs=sgn_sb[:], start=True, stop=True)
                sgn_bc = cpool.tile([128, F], bf)
                nc.vector.tensor_copy(out=sgn_bc[:], in_=ps2[:])
                ga_bc, _ = bcast_f32(lnga, 256, "ga")
                ba_bc, _ = bcast_f32(lnba, 256, "ba")
                g1_bc, _ = bcast_f32(lng1, 128, "g1")
                b1_bc, _ = bcast_f32(lnb1, 128, "b1")
                g2_bc, _ = bcast_f32(lng2, 64, "g2")
                b2_bc, _ = bcast_f32(lnb2, 64, "b2")

            # ---------- Phase A1: full xl table ----------
            wl_sb = cpool.tile([128, 4, F], bf)
            for k in range(4):
                nc.sync.dma_start(out=wl_sb[:, k, :], in_=wl[k * 128:(k + 1) * 128, :])
            bl_sb = cpool.tile([1, F], bf)
            nc.sync.dma_start(out=bl_sb[:], in_=blr[:])
            wr_sb = cpool.tile([128, 4, F], bf)
            for k in range(4):
                nc.sync.dma_start(out=wr_sb[:, k, :], in_=wr[k * 128:(k + 1) * 128, :])
            br_sb = cpool.tile([1, F], bf)
            nc.sync.dma_start(out=br_sb[:], in_=brr[:])

            xT_v = xT[:].rearrange("(a p) n -> p a n", p=128)
            with tc.tile_pool(name="a1x", bufs=4) as xa_pool, \
                 tc.tile_pool(name="a1ps", bufs=4, space="PSUM") as ps_a, \
                 tc.tile_pool(name="a1bld", bufs=4) as bld_pool:
                for m in range(NM):
                    xa = xa_pool.tile([128, 4, 128], bf)
                    nc.sync.dma_start(out=xa[:], in_=xT_v[:, :, m * 128:(m + 1) * 128])
                    ps = ps_a.tile([128, F], f32)
                    for k in range(4):
                        nc.tensor.matmul(out=ps[:], lhsT=xa[:, k, :], rhs=wl_sb[:, k, :],
                                         start=(k == 0), stop=False)
                    nc.tensor.matmul(out=ps[:], lhsT=ones1[:], rhs=bl_sb[:], start=False, stop=True)
                    bld = bld_pool.tile([128, 514], bf)
                    nc.vector.memset(bld[:, 256:257], 1.0)
                    nc.vector.memset(bld[:, 513:514], 1.0)
                    nc.vector.tensor_copy(out=bld[:, 0:256], in_=ps[:, 0:256])
                    nc.vector.tensor_copy(out=bld[:, 257:513], in_=ps[:, 256:512])
                    nc.sync.dma_start(out=table[m * 128:(m + 1) * 128, :], in_=bld[:])

            # ---------- Phase A2: local xr (node-major, SBUF-resident) ----------
            vb_pool = ctx.enter_context(tc.tile_pool(name="vb", bufs=NB))
            vbs = []
            xTl_v = xTl[:].rearrange("(a p) n -> p a n", p=128)
            with tc.tile_pool(name="a2x", bufs=2) as xa2_pool, \
                 tc.tile_pool(name="a2ps", bufs=2, space="PSUM") as ps_a2:
                for b in range(NB):
                    xa = xa2_pool.tile([128, 4, 128], bf)
                    nc.sync.dma_start(out=xa[:], in_=xTl_v[:, :, b * 128:(b + 1) * 128])
                    ps = ps_a2.tile([128, F], f32)
                    for k in range(4):
                        nc.tensor.matmul(out=ps[:], lhsT=xa[:, k, :], rhs=wr_sb[:, k, :],
                                         start=(k == 0), stop=False)
                    nc.tensor.matmul(out=ps[:], lhsT=ones1[:], rhs=br_sb[:], start=False, stop=True)
                    vb = vb_pool.tile([128, F], bf, tag="vb", bufs=NB)
                    nc.scalar.activation(out=vb[:], in_=ps[:], func=AF.Copy)
                    vbs.append(vb)

            # h1T storage (transposed GAT output, bf16, persistent)
            h1T_pool = ctx.enter_context(tc.tile_pool(name="h1T", bufs=NB))
            h1Ts = []

            if phases != "a":
                # ---------- Phase B: edge loop ----------
                # Two passes per block to avoid per-tile cross-engine
                # round-trips (strict-FIFO engine queues serialize on them):
                #   pass 1 (all tiles): gather -> pm matmuls -> scalar Prelu
                #     -> DVE signed-sum accums into ecb
                #   one block-wide Exp
                #   pass 2 (all tiles): sz one-hots (DVE + scalar) -> U matmuls
                with tc.tile_pool(name="m01p", bufs=2) as m01_pool, \
                     tc.tile_pool(name="mtp", bufs=2) as mt_pool, \
                     tc.tile_pool(name="sip", bufs=2) as si_pool, \
                     tc.tile_pool(name="xgp", bufs=TB + 3) as xg_pool, \
                     tc.tile_pool(name="pmp", bufs=3, space="PSUM") as pm_pool, \
                     tc.tile_pool(name="Up", bufs=2, space="PSUM") as U_pool, \
                     tc.tile_pool(name="scrp", bufs=3) as scr_pool, \
                     tc.tile_pool(name="ep", bufs=2) as e_pool, \
                     tc.tile_pool(name="szp", bufs=3) as sz_pool, \
                     tc.tile_pool(name="h1p", bufs=2) as h1_pool, \
                     tc.tile_pool(name="h1ps", bufs=2, space="PSUM") as h1ps_pool:
                    for b in range(NB):
                        m01_sb = m01_pool.tile([128, TB, 128], bf)
                        nc.sync.dma_start(out=m01_sb[:], in_=m01_in[b])
                        mt_sb = mt_pool.tile([128, TB, 128], bf)
                        nc.sync.dma_start(out=mt_sb[:], in_=mt_in[b])
                        edl_sb = si_pool.tile([128, TB], f32, tag="edl")
                        nc.sync.dma_start(out=edl_sb[:], in_=edl_in[b])
                        si_sb = si_pool.tile([128, TB], dt.int32, tag="si")
                        nc.sync.dma_start(out=si_sb[:], in_=sidx[b])
                        U0 = U_pool.tile([128, 257], f32, tag="U")
                        U1 = U_pool.tile([128, 257], f32, tag="U")
                        ecb = e_pool.tile([128, TB, 2], f32, tag="ecb")
                        xgs = []
                        for t in range(TB):
                            xg = xg_pool.tile([128, 514], bf, bufs=TB + 3)
                            nc.gpsimd.indirect_dma_start(
                                out=xg[:], out_offset=None, in_=table[:],
                                in_offset=bass.IndirectOffsetOnAxis(
                                    ap=si_sb[:, t:t + 1], axis=0))
                            xgs.append(xg)
                        if phases == "gather":
                            continue
                        # pass 1
                        for t in range(TB):
                            pm = pm_pool.tile([128, F], f32)
                            xg_m = xgs[t][:].rearrange("p (h x) -> p h x", x=257)[:, :, 0:256]
                            nc.tensor.matmul(out=pm[:], lhsT=ident[:], rhs=xg_m, start=True, stop=False)
                            nc.tensor.matmul(out=pm[:], lhsT=m01_sb[:, t, :], rhs=vbs[b][:],
                                             start=False, stop=True)
                            scr = scr_pool.tile([128, F], bf, tag="scr")
                            scr2 = scr_pool.tile([128, F], bf, tag="scr2")
                            # scr = lrelu(p) over all 512 cols (single PSUM pass)
                            nc.scalar.activation(out=scr[:], in_=pm[:],
                                                 func=AF.Prelu, alpha=0.2)
                            # e_h = sum(scr * sign) per head: one stt accum each
                            nc.vector.scalar_tensor_tensor(
                                out=scr2[:, 0:256], in0=scr[:, 0:256], scalar=1.0,
                                in1=sgn_bc[:, 0:256], op0=ALU.mult, op1=ALU.mult,
                                accum_out=ecb[:, t, 0:1])
                            nc.vector.scalar_tensor_tensor(
                                out=scr2[:, 256:512], in0=scr[:, 256:512], scalar=1.0,
                                in1=sgn_bc[:, 256:512], op0=ALU.mult, op1=ALU.mult,
                                accum_out=ecb[:, t, 1:2])
                        # block-wide exp
                        zb = e_pool.tile([128, TB, 2], f32, tag="zb")
                        nc.scalar.activation(out=zb[:], in_=ecb[:], func=AF.Exp)
                        # pass 2
                        for t in range(TB):
                            sz = sz_pool.tile([128, 2, 128], bf)
                            nc.vector.tensor_scalar(out=sz[:, 0, :], in0=iota_sb[:],
                                                    scalar1=edl_sb[:, t:t + 1],
                                                    scalar2=zb[:, t, 0:1],
                                                    op0=ALU.is_equal, op1=ALU.mult)
                            nc.scalar.activation(out=sz[:, 1, :], in_=mt_sb[:, t, :],
                                                 func=AF.Copy, scale=zb[:, t, 1:2])
                            nc.tensor.matmul(out=U0[:], lhsT=sz[:, 0, :], rhs=xgs[t][:, 0:257],
                                             start=(t == 0), stop=(t == TB - 1))
                            nc.tensor.matmul(out=U1[:], lhsT=sz[:, 1, :], rhs=xgs[t][:, 257:514],
                                             start=(t == 0), stop=(t == TB - 1))
                        # h1 assembly for block b
                        rc = e_pool.tile([128, 2], f32, tag="rc")
                        nc.vector.reciprocal(out=rc[:, 0:1], in_=U0[:, 256:257])
                        nc.vector.reciprocal(out=rc[:, 1:2], in_=U1[:, 256:257])
                        h1a = h1_pool.tile([128, F], f32, tag="h1a")
                        nc.vector.tensor_scalar(out=h1a[:, 0:256], in0=U0[:, 0:256],
                                                scalar1=rc[:, 0:1], scalar2=None, op0=ALU.mult)
                        nc.vector.tensor_scalar(out=h1a[:, 256:512], in0=U1[:, 0:256],
                                                scalar1=rc[:, 1:2], scalar2=None, op0=ALU.mult)
                        h1b = h1_pool.tile([128, F], f32, tag="h1b")
                        nc.vector.tensor_tensor(out=h1b[:], in0=h1a[:], in1=biasc_bc[:], op=ALU.add)
                        h1f = h1_pool.tile([128, F], bf, tag="h1f")
                        nc.scalar.activation(out=h1f[:], in_=h1b[:], func=AF.Relu)
                        h1T_ps = h1ps_pool.tile([128, F], bf, tag="h1T_ps")
                        for k in range(4):
                            nc.tensor.transpose(out=h1T_ps[:, k * 128:(k + 1) * 128],
                                                in_=h1f[:, k * 128:(k + 1) * 128],
                                                identity=ident[:])
                        h1T = h1T_pool.tile([128, 4, 128], bf, tag="h1T", bufs=NB)
                        nc.vector.tensor_copy(out=h1T[:], in_=h1T_ps[:])
                        h1Ts.append(h1T)

            if phases == "full":
                # ---------- Phase C: MLP head ----------
                wda_sb = cpool.tile([128, 4, 256], bf)
                for k in range(4):
                    nc.sync.dma_start(out=wda_sb[:, k, :], in_=wda[k * 128:(k + 1) * 128, :])
                wala_sb = cpool.tile([128, 4, 256], bf)
                for k in range(4):
                    nc.sync.dma_start(out=wala_sb[:, k, :], in_=wala[k * 128:(k + 1) * 128, :])
                wd1_sb = cpool.tile([128, 2, 128], bf)
                for k in range(2):
                    nc.sync.dma_start(out=wd1_sb[:, k, :], in_=wd1[k * 128:(k + 1) * 128, :])
                wal1_sb = cpool.tile([128, 2, 128], bf)
                for k in range(2):
                    nc.sync.dma_start(out=wal1_sb[:, k, :], in_=wal1[k * 128:(k + 1) * 128, :])
                wd2_sb = cpool.tile([128, 64], bf)
                nc.sync.dma_start(out=wd2_sb[:], in_=wd2[:])
                wd3_sb = cpool.tile([64, 3], bf)
                nc.sync.dma_start(out=wd3_sb[:], in_=wd3[:])
                bda_sb = cpool.tile([1, 256], bf)
                nc.sync.dma_start(out=bda_sb[:], in_=bda[:])
                bala_sb = cpool.tile([1, 256], bf)
                nc.sync.dma_start(out=bala_sb[:], in_=bala[:])
                bd1_sb = cpool.tile([1, 128], bf)
                nc.sync.dma_start(out=bd1_sb[:], in_=bd1[:])
                bal1_sb = cpool.tile([1, 128], bf)
                nc.sync.dma_start(out=bal1_sb[:], in_=bal1[:])
                bd2_sb = cpool.tile([1, 64], bf)
                nc.sync.dma_start(out=bd2_sb[:], in_=bd2[:])
                bd3_sb = cpool.tile([1, 3], bf)
                nc.sync.dma_start(out=bd3_sb[:], in_=bd3[:])

                yaT_sb = cpool.tile([AGW, ND], bf)      # rhs-side aug
                laT_sb = cpool.tile([AGW, ND], bf)      # lhsT-side aug
                sqb_pool = ctx.enter_context(tc.tile_pool(name="sqb", bufs=NB))
                sqbs = []

                def layer_norm_affine(tc_ps, width, g_bc, b_bc, sm_pool):
                    """Returns z_norm*g + b in SBUF (f32) from psum z [128, width]."""
                    stats = sm_pool.tile([128, 6], f32, tag="stats")
                    nc.vector.bn_stats(out=stats[:], in_=tc_ps[:])
                    mv = sm_pool.tile([128, 2], f32, tag="mv")
                    nc.vector.bn_aggr(out=mv[:], in_=stats[:])
                    sd = sm_pool.tile([128, 1], f32, tag="sd")
                    nc.scalar.activation(out=sd[:], in_=mv[:, 1:2], func=AF.Sqrt, bias=eps_sb[:, 0:1])
                    rs = sm_pool.tile([128, 1], f32, tag="rs")
                    nc.vector.reciprocal(out=rs[:], in_=sd[:])
                    nmr = sm_pool.tile([128, 1], f32, tag="nmr")
                    nc.vector.tensor_scalar(out=nmr[:], in0=mv[:, 0:1], scalar1=rs[:, 0:1],
                                            scalar2=-1.0, op0=ALU.mult, op1=ALU.mult)
                    zn = sm_pool.tile([128, width], f32, tag="zn")
                    nc.scalar.activation(out=zn[:], in_=tc_ps[:], func=AF.Identity,
                                         bias=nmr[:, 0:1], scale=rs[:, 0:1])
                    zg = sm_pool.tile([128, width], f32, tag="zg")
                    nc.vector.tensor_tensor(out=zg[:], in0=zn[:], in1=g_bc[:, 0:width], op=ALU.mult)
                    zb = sm_pool.tile([128, width], f32, tag="zb")
                    nc.vector.tensor_tensor(out=zb[:], in0=zg[:], in1=b_bc[:, 0:width], op=ALU.add)
                    return zb

                with tc.tile_pool(name="mlpps", bufs=2, space="PSUM") as mps, \
                     tc.tile_pool(name="mlpsm", bufs=4) as sm_pool, \
                     tc.tile_pool(name="mlpsb", bufs=4) as msb:
                    for b in range(NB):
                        h1T = h1Ts[b]
                        # layer a: za = h1@Wd_a + bd_a ; res = h1@Wal_a + bal_a
                        za = mps.tile([128, 256], f32, tag="mm0")
                        for k in range(4):
                            nc.tensor.matmul(out=za[:], lhsT=h1T[:, k, :], rhs=wda_sb[:, k, :],
                                             start=(k == 0), stop=False)
                        nc.tensor.matmul(out=za[:], lhsT=ones1[:], rhs=bda_sb[:], start=False, stop=True)
                        res = mps.tile([128, 256], f32, tag="mm1")
                        for k in range(4):
                            nc.tensor.matmul(out=res[:], lhsT=h1T[:, k, :], rhs=wala_sb[:, k, :],
                                             start=(k == 0), stop=False)
                        nc.tensor.matmul(out=res[:], lhsT=ones1[:], rhs=bala_sb[:], start=False, stop=True)
                        zb = layer_norm_affine(za, 256, ga_bc, ba_bc, sm_pool)
                        zr = msb.tile([128, 256], f32, tag="zr")
                        nc.scalar.activation(out=zr[:], in_=zb[:], func=AF.Relu)
                        h2 = msb.tile([128, 256], bf, tag="h2")
                        nc.vector.tensor_tensor(out=h2[:], in0=zr[:], in1=res[:], op=ALU.add)
                        h2T_ps = mps.tile([128, 256], bf, tag="tp")
                        for k in range(2):
                            nc.tensor.transpose(out=h2T_ps[:, k * 128:(k + 1) * 128],
                                                in_=h2[:, k * 128:(k + 1) * 128], identity=ident[:])
                        h2T = msb.tile([128, 2, 128], bf, tag="h2Ts")
                        nc.vector.tensor_copy(out=h2T[:], in_=h2T_ps[:])
                        # layer 1
                        z1 = mps.tile([128, 128], f32, tag="mm0")
                        for k in range(2):
                            nc.tensor.matmul(out=z1[:], lhsT=h2T[:, k, :], rhs=wd1_sb[:, k, :],
                                             start=(k == 0), stop=False)
                        nc.tensor.matmul(out=z1[:], lhsT=ones1[:], rhs=bd1_sb[:], start=False, stop=True)
                        res1 = mps.tile([128, 128], f32, tag="mm1")
                        for k in range(2):
                            nc.tensor.matmul(out=res1[:], lhsT=h2T[:, k, :], rhs=wal1_sb[:, k, :],
                                             start=(k == 0), stop=False)
                        nc.tensor.matmul(out=res1[:], lhsT=ones1[:], rhs=bal1_sb[:], start=False, stop=True)
                        zb1 = layer_norm_affine(z1, 128, g1_bc, b1_bc, sm_pool)
                        zr1 = msb.tile([128, 128], f32, tag="zr1")
                        nc.scalar.activation(out=zr1[:], in_=zb1[:], func=AF.Relu)
                        h3 = msb.tile([128, 128], bf, tag="h3")
                        nc.vector.tensor_tensor(out=h3[:], in0=zr1[:], in1=res1[:], op=ALU.add)
                        h3T_ps = mps.tile([128, 128], bf, tag="tp")
                        nc.tensor.transpose(out=h3T_ps[:], in_=h3[:], identity=ident[:])
                        h3T = msb.tile([128, 128], bf, tag="h3Ts")
                        nc.vector.tensor_copy(out=h3T[:], in_=h3T_ps[:])
                        # layer 2 (no residual)
                        z2m = mps.tile([128, 64], f32, tag="mm0")
                        nc.tensor.matmul(out=z2m[:], lhsT=h3T[:], rhs=wd2_sb[:], start=True, stop=False)
                        nc.tensor.matmul(out=z2m[:], lhsT=ones1[:], rhs=bd2_sb[:], start=False, stop=True)
                        zb2 = layer_norm_affine(z2m, 64, g2_bc, b2_bc, sm_pool)
                        h4 = msb.tile([128, 64], bf, tag="h4")
                        nc.scalar.activation(out=h4[:], in_=zb2[:], func=AF.Relu)
                        h4T_ps = mps.tile([64, 128], bf, tag="tp")
                        nc.tensor.transpose(out=h4T_ps[:], in_=h4[:, :64], identity=ident[:])
                        h4T = msb.tile([64, 128], bf, tag="h4Ts")
                        nc.vector.tensor_copy(out=h4T[:], in_=h4T_ps[:])
                        # y = h4 @ Wd3 + bd3
                        y_ps = mps.tile([128, 3], f32, tag="mm0")
                        nc.tensor.matmul(out=y_ps[:], lhsT=h4T[:], rhs=wd3_sb[:], start=True, stop=False)
                        nc.tensor.matmul(out=y_ps[:], lhsT=ones1[:], rhs=bd3_sb[:], start=False, stop=True)
                        yf = msb.tile([128, 3], f32, tag="yf")
                        nc.vector.tensor_copy(out=yf[:], in_=y_ps[:])
                        nc.sync.dma_start(out=y_out[b * 128:(b + 1) * 128, :], in_=yf[:])
                        # hi/lo split of y and sq for fp16 cdist aug
                        yh = msb.tile([128, 3], bf, tag="yh")
                        nc.vector.tensor_copy(out=yh[:], in_=y_ps[:])
                        ylf = msb.tile([128, 3], f32, tag="ylf")
                        nc.vector.tensor_tensor(out=ylf[:], in0=y_ps[:], in1=yh[:], op=ALU.subtract)
                        yl = msb.tile([128, 3], bf, tag="yl")
                        nc.vector.tensor_copy(out=yl[:], in_=ylf[:])
                        ysq = msb.tile([128, 3], f32, tag="ysq")
                        sq = msb.tile([128, 1], f32, tag="sq")
                        nc.scalar.activation(out=ysq[:], in_=y_ps[:], func=AF.Square, accum_out=sq[:])
                        sqh = msb.tile([128, 1], bf, tag="sqh")
                        nc.vector.tensor_copy(out=sqh[:], in_=sq[:])
                        sqlf = msb.tile([128, 1], f32, tag="sqlf")
                        nc.vector.tensor_tensor(out=sqlf[:], in0=sq[:], in1=sqh[:], op=ALU.subtract)
                        sql = msb.tile([128, 1], bf, tag="sql")
                        nc.vector.tensor_copy(out=sql[:], in_=sqlf[:])
                        # sqb = sq + EPS2 (per-partition bias for phase D sqrt)
                        sqb = sqb_pool.tile([128, 1], f32, tag="sqb", bufs=NB)
                        nc.vector.tensor_scalar(out=sqb[:], in0=sq[:], scalar1=EPS2,
                                                scalar2=None, op0=ALU.add)
                        sqbs.append(sqb)
                        # aug vectors (slot pairing):
                        #  ya = [sqh, sql, yh(3), yl(3), yh(3)]
                        #  la = [1,   1,   -2yh(3), -2yh(3), -2yl(3)]
                        m2yh = msb.tile([128, 3], bf, tag="m2yh")
                        nc.scalar.activation(out=m2yh[:], in_=yh[:], func=AF.Copy, scale=-2.0)
                        m2yl = msb.tile([128, 3], bf, tag="m2yl")
                        nc.scalar.activation(out=m2yl[:], in_=yl[:], func=AF.Copy, scale=-2.0)
                        ya = msb.tile([128, AGW], bf, tag="ya")
                        nc.vector.tensor_copy(out=ya[:, 0:1], in_=sqh[:])
                        nc.vector.tensor_copy(out=ya[:, 1:2], in_=sql[:])
                        nc.vector.tensor_copy(out=ya[:, 2:5], in_=yh[:])
                        nc.vector.tensor_copy(out=ya[:, 5:8], in_=yl[:])
                        nc.vector.tensor_copy(out=ya[:, 8:11], in_=yh[:])
                        la = msb.tile([128, AGW], bf, tag="la")
                        nc.vector.memset(la[:, 0:2], 1.0)
                        nc.vector.tensor_copy(out=la[:, 2:5], in_=m2yh[:])
                        nc.vector.tensor_copy(out=la[:, 5:8], in_=m2yh[:])
                        nc.vector.tensor_copy(out=la[:, 8:11], in_=m2yl[:])
                        yaT_ps = mps.tile([AGW, 128], bf, tag="tp")
                        nc.tensor.transpose(out=yaT_ps[:], in_=ya[:], identity=ident[:])
                        nc.vector.tensor_copy(out=yaT_sb[:, b * 128:(b + 1) * 128], in_=yaT_ps[:])
                        laT_ps = mps.tile([AGW, 128], bf, tag="tp")
                        nc.tensor.transpose(out=laT_ps[:], in_=la[:], identity=ident[:])
                        nc.vector.tensor_copy(out=laT_sb[:, b * 128:(b + 1) * 128], in_=laT_ps[:])

                # ---------- AllGather y augmentation ----------
                nc.sync.dma_start(out=cc_in[:], in_=yaT_sb[:])
                nc.gpsimd.collective_compute(
                    "AllGather", ALU.bypass,
                    replica_groups=[list(range(NC))],
                    ins=[cc_in[:].opt()],
                    outs=[cc_out[:].opt()],
                )
                ag_tiles = []
                for c in range(NC):
                    agt = cpool.tile([AGW, ND], bf, name=f"ag{c}")
                    nc.sync.dma_start(out=agt[:], in_=cc_out[AGW * c:AGW * (c + 1), :])
                    ag_tiles.append(agt)

                # ---------- Phase D: cdist ----------
                # 2-chunk batches: two N=512 matmuls into one 2-bank psum,
                # then a single sqrt pass + single 256KB DMA per kilo-chunk.
                CH = N // 512   # 24 column chunks of 512
                with tc.tile_pool(name="dps", bufs=3, space="PSUM") as dps, \
                     tc.tile_pool(name="dsb", bufs=3) as dsb:
                    for m in range(NB):
                        for chp in range(CH // 2):
                            d2 = dps.tile([128, 1024], f32, tag="d2")
                            for half in range(2):
                                cidx = (chp * 2 + half) * 512
                                cc = cidx % ND
                                nc.tensor.matmul(out=d2[:, half * 512:(half + 1) * 512],
                                                 lhsT=laT_sb[:, m * 128:(m + 1) * 128],
                                                 rhs=ag_tiles[cidx // ND][:, cc:cc + 512],
                                                 start=True, stop=True)
                            dsq = dsb.tile([128, 1024], bf, tag="dsq")
                            nc.scalar.activation(out=dsq[:], in_=d2[:], func=AF.Sqrt,
                                                 bias=sqbs[m][:, 0:1])
                            cidx = chp * 1024
                            nc.sync.dma_start(out=out[m * 128:(m + 1) * 128, cidx:cidx + 1024],
                                              in_=dsq[:])
    nc.compile()
    return nc


# ---------------- host preprocessing ----------------

def host_prep(inputs):
    bf16 = np.float16
    x = np.asarray(inputs["x"], np.float32)
    ei = np.asarray(inputs["edge_index"])
    src = np.concatenate([ei[0], np.arange(N)]).astype(np.int32)
    dst = np.concatenate([ei[1], np.arange(N)]).astype(np.int32)
    order = np.argsort(dst, kind="stable")
    src, dst = src[order], dst[order]
    blk = dst // 128
    counts = np.bincount(blk, minlength=NC * NB)
    assert counts.max() <= TB * 128, f"block overflow: {counts.max()}"
    starts = np.zeros(NC * NB + 1, np.int64)
    np.cumsum(counts, out=starts[1:])

    # |att| column scaling folded into the GAT linear layers (inverted in
    # the first MLP layer); att signs shipped as a +-1 row for the DVE
    # signed reduction.
    att = np.asarray(inputs["att"], np.float64).reshape(-1)   # [512]
    colscale = np.maximum(np.abs(att), 1e-12)
    sgn_row = np.where(att > 0, 1.0, -1.0).astype(np.float32)

    def f64(name):
        return np.asarray(inputs[name], np.float64)

    Wl_p = (f64("Wl") * colscale[None, :]).astype(np.float32)
    bl_p = (f64("bl") * colscale).astype(np.float32)
    Wr_p = (f64("Wr") * colscale[None, :]).astype(np.float32)
    br_p = (f64("br") * colscale).astype(np.float32)
    biasc_p = (f64("bias_c") * colscale).astype(np.float32)
    inv = 1.0 / colscale
    Wda_p = (f64("Wd_a") * inv[:, None]).astype(np.float32)
    Wala_p = (f64("Wal_a") * inv[:, None]).astype(np.float32)

    xT_b = np.ascontiguousarray(x.T).astype(bf16)

    def b16(arr):
        return np.ascontiguousarray(np.asarray(arr, np.float32)).astype(bf16)

    def row16(arr):
        return np.asarray(arr, np.float32)[None, :].astype(bf16)

    def rowf(name):
        return np.ascontiguousarray(np.asarray(inputs[name], np.float32)[None, :])

    shared = dict(
        xT=xT_b,
        wl=b16(Wl_p), blr=row16(bl_p),
        wr=b16(Wr_p), brr=row16(br_p),
        biasc=row16(biasc_p),
        sgn=row16(sgn_row),
        wda=b16(Wda_p), bda=row16(inputs["bd_a"]),
        wala=b16(Wala_p), bala=row16(inputs["bal_a"]),
        wd1=b16(inputs["Wd1"]), bd1=row16(inputs["bd1"]),
        wal1=b16(inputs["Wal1"]), bal1=row16(inputs["bal1"]),
        wd2=b16(inputs["Wd2"]), bd2=row16(inputs["bd2"]),
        wd3=b16(inputs["Wd3"]), bd3=row16(inputs["bd3"]),
        lnga=rowf("ga"), lnba=rowf("bta"),
        lng1=rowf("g1"), lnb1=rowf("bt1"),
        lng2=rowf("g2"), lnb2=rowf("bt2"),
    )
    in_maps = []
    for c in range(NC):
        si = np.zeros((NB, 128, TB), np.int32)
        ed = np.full((NB, 128, TB), 255.0, np.float32)
        m01 = np.zeros((NB, 128, TB, 128), bf16)
        mt = np.zeros((NB, 128, TB, 128), bf16)
        for b in range(NB):
            g = c * NB + b
            s, e = starts[g], starts[g + 1]
            cnt = e - s
            esrc = src[s:e]
            edl = (dst[s:e] - (g * 128)).astype(np.int32)
            pad = TB * 128 - cnt
            esrc = np.concatenate([esrc, np.zeros(pad, np.int32)])
            edl = np.concatenate([edl, np.full(pad, 255, np.int32)])
            esrc = esrc.reshape(TB, 128)
            edl = edl.reshape(TB, 128)
            si[b] = esrc.T
            ed[b] = edl.T.astype(np.float32)
            onehot = (edl[:, :, None] == np.arange(128)[None, None, :])
            m01[b] = onehot.transpose(2, 0, 1).astype(bf16)
            mt[b] = onehot.transpose(1, 0, 2).astype(bf16)
        m = dict(shared)
        m["tok"] = np.zeros((1, 8), np.float32)
        m["xTl"] = np.ascontiguousarray(xT_b[:, c * ND:(c + 1) * ND])
        m["sidx"] = si
        m["edl_in"] = ed
        m["m01_in"] = m01
        m["mt_in"] = mt
        in_maps.append(m)
    return in_maps


class _Runner:
    """Compile once; keep a reusable jitted sharded executable.

    Mirrors concourse.bass2jax.run_bass_via_pjrt's multi-core path, but
    memoizes the jit so repeated calls don't re-trace, and exposes a
    device-resident timing mode.
    """

    def __init__(self, phases="full", repeats=1):
        import jax
        import concourse.mybir as mb
        from concourse import bass2jax
        from jax.sharding import Mesh, PartitionSpec, NamedSharding
        from jax.experimental.shard_map import shard_map

        bass2jax.install_neuronx_cc_hook()
        nc = build_program(phases, repeats)
        self.nc = nc
        part_name = nc.partition_id_tensor.name if nc.partition_id_tensor else None
        in_names, out_names, out_avals, zero_shapes = [], [], [], []
        for alloc in nc.m.functions[0].allocations:
            if not isinstance(alloc, mb.MemoryLocationSet):
                continue
            name = alloc.memorylocations[0].name
            if alloc.kind == "ExternalInput":
                if name != part_name:
                    in_names.append(name)
            elif alloc.kind == "ExternalOutput":
                out_names.append(name)
                out_avals.append(jax.core.ShapedArray(
                    tuple(alloc.tensor_shape), mb.dt.np(alloc.dtype)))
                zero_shapes.append((tuple(alloc.tensor_shape), mb.dt.np(alloc.dtype)))
        n_params = len(in_names)
        n_outs = len(out_names)
        all_names = in_names + out_names
        if part_name is not None:
            all_names = all_names + [part_name]
        self.in_names = in_names
        self.out_names = out_names

        def _body(*args):
            operands = list(args)
            if part_name is not None:
                operands.append(bass2jax.partition_id_tensor())
            outs = bass2jax._bass_exec_p.bind(
                *operands,
                out_avals=tuple(out_avals),
                in_names=tuple(all_names),
                out_names=tuple(out_names),
                lowering_input_output_aliases=(),
                sim_require_finite=True,
                sim_require_nnan=True,
                nc=nc,
            )
            return tuple(outs)

        devices = jax.devices()[:NC]
        mesh = Mesh(np.asarray(devices), ("core",))
        self.mesh = mesh
        self.sharding = NamedSharding(mesh, PartitionSpec("core"))
        in_specs = (PartitionSpec("core"),) * (n_params + n_outs)
        out_specs = (PartitionSpec("core"),) * n_outs
        # outputs are fully written by the kernel, so no donation / zero-init
        # is needed; the zero args exist only to satisfy the parameter list.
        self.sharded = jax.jit(
            shard_map(_body, mesh=mesh, in_specs=in_specs, out_specs=out_specs,
                      check_rep=False),
            keep_unused=True)

        tok_in_idx = in_names.index("tok")
        tok_out_idx = out_names.index("tok_out")

        def _bodyK(K):
            def f(*args):
                args = list(args)
                outs = None
                for _ in range(K):
                    outs = _body(*args)
                    args[tok_in_idx] = outs[tok_out_idx]
                return tuple(outs)
            return f

        self._mk_chain = lambda K: jax.jit(
            shard_map(_bodyK(K), mesh=mesh, in_specs=in_specs,
                      out_specs=out_specs, check_rep=False),
            keep_unused=True)
        self._chains = {}

        import jax.numpy as jnp

        def _zeros():
            return tuple(jnp.zeros((NC * s[0], *s[1:]), d) for s, d in zero_shapes)

        self.zmaker = jax.jit(_zeros, out_shardings=(self.sharding,) * n_outs)
        self._zs = None
        self.jax = jax

    def put_inputs(self, in_maps):
        concat = [np.concatenate([np.asarray(in_maps[c][k]) for c in range(NC)], axis=0)
                  for k in self.in_names]
        return [self.jax.device_put(a, self.sharding) for a in concat]

    def zeros(self):
        if self._zs is None:
            self._zs = self.zmaker()
            self.jax.block_until_ready(self._zs)
        return self._zs

    def exec_dev(self, dev_inputs):
        outs = self.sharded(*dev_inputs, *self.zeros())
        self.jax.block_until_ready(outs)
        return outs

    def exec_chain(self, dev_inputs, K):
        if K not in self._chains:
            self._chains[K] = self._mk_chain(K)
        outs = self._chains[K](*dev_inputs, *self.zeros())
        self.jax.block_until_ready(outs)
        return outs

    def run(self, in_maps):
        dev_inputs = self.put_inputs(in_maps)
        outs = self.exec_dev(dev_inputs)
        res = []
        for c in range(NC):
            d = {}
            for i, name in enumerate(self.out_names):
                arr = np.asarray(outs[i])
                d[name] = arr.reshape(NC, -1, *arr.shape[1:])[c].reshape(
                    arr.shape[0] // NC, *arr.shape[1:])
            res.append(d)
        return res


def _get_runner(phases="full", repeats=1):
    key = f"runner_{phases}_{repeats}"
    if key not in _cache:
        _cache[key] = _Runner(phases, repeats)
    return _cache[key]


def kernel(**inputs):
    in_maps = host_prep(inputs)
    runner = _get_runner()
    results = runner.run(in_maps)
    out = np.concatenate([results[c]["out"] for c in range(NC)], axis=0)
    return out.astype(np.float32)
